# revision 1
# baseline (speedup 1.0000x reference)
"""AxialRoPE self-attention on 8 Trainium2 NeuronCores.

Sharding: 8 cores = 4 batches x 2 head-groups (8 heads each).
Each core computes q/k/v projections for its head-group over the full
sequence of its batch, RoPE, attention, and a partial output projection
(row-sharded Wo). Host sums the two partial outputs per batch.

Per-core kernel (all matmuls bf16 with fp32 PSUM accumulation):
  xT [1024, 2048] = x[b]^T (host-prepped bf16)
  QT = Wq^T x^T + bq   [512, 2048] head-dim-major; same for K; V natural.
  RoPE: qt' = QT*cosT + shift(QT)*sinTs where shift swaps partition pairs
  (2i <-> 2i+1) via two partition-strided SBUF DMAs and sinTs carries the
  (-1)^(d+1) sign.
  scoresT[ks, qs] per head, 2 heads packed in the PE array (K=64 row tiles),
  both heads' scores in one [128, 2048] psum tile -> single exp (scalar
  engine, scale=1/8, no max subtraction: scores are ~N(0,1), max < 7).
  PV: lhsT = V_aug [ks, 65] (65th column of ones -> row 64 = softmax
  denominator). Normalization: recip = exp(-ln(sum)) on ACT, broadcast via
  a K=1 ones matmul, applied on the DVE.
"""

import os
import numpy as np

B, S, D = 4, 2048, 1024
NHEAD, HDIM = 16, 64
HG = 2                # head-group shards
HPC = NHEAD // HG     # 8 heads per core
DG = HPC * HDIM       # 512 local projection width
NCORES = 8
ROPE_BASE = 10000.0

_CACHE = {}


def _build_program():
    from concourse import bass, bacc, tile
    from concourse import mybir

    dt = mybir.dt
    f32, bf16 = dt.float32, dt.bfloat16
    AF = mybir.ActivationFunctionType
    ALU = mybir.AluOpType
    PSUM = bass.MemorySpace.PSUM

    nc = bacc.Bacc("TRN2", target_bir_lowering=False, debug=False)

    # The PJRT-side NEFF cache keys on the HLO signature, which sees only
    # tensor shapes -- encode a build nonce in a dummy input's shape so
    # program variants with identical I/O still recompile.
    _nw = (int(os.environ.get("BUILD_REPEAT", "1"))
           + 100 * int(os.environ.get("BUILD_NONCE", "0")))
    nc.dram_tensor("nonce", [1, _nw], f32, kind="ExternalInput")

    xT_d = nc.dram_tensor("xT", [D, S], bf16, kind="ExternalInput")
    wq_d = nc.dram_tensor("wq", [D, DG], bf16, kind="ExternalInput")
    wk_d = nc.dram_tensor("wk", [D, DG], bf16, kind="ExternalInput")
    wv_d = nc.dram_tensor("wv", [D, DG], bf16, kind="ExternalInput")
    wo_d = nc.dram_tensor("wo", [DG, D], bf16, kind="ExternalInput")
    cos_d = nc.dram_tensor("cosT", [128, S], bf16, kind="ExternalInput")
    sin_d = nc.dram_tensor("sinTs", [128, S], bf16, kind="ExternalInput")
    bq_d = nc.dram_tensor("bq4", [128, 4], f32, kind="ExternalInput")
    bk_d = nc.dram_tensor("bk4", [128, 4], f32, kind="ExternalInput")
    bv_d = nc.dram_tensor("bv", [1, DG], bf16, kind="ExternalInput")
    bo_d = nc.dram_tensor("bo", [1, D], bf16, kind="ExternalInput")
    out_d = nc.dram_tensor("out", [S, D], f32, kind="ExternalOutput")

    CC = D // 128    # 8 contraction chunks
    DC = DG // 128   # 4 dout chunks (2 heads each)
    SC = S // 128    # 16 sequence chunks
    QT2 = 512        # query tile
    NQ = S // QT2    # 4
    NPP = NQ * 2     # denominator points per pair

    with tile.TileContext(nc) as tc:
        with (
            tc.tile_pool(name="persist", bufs=1) as P,
            tc.tile_pool(name="ps_all", bufs=2, space=PSUM) as PS,
            tc.tile_pool(name="tmp", bufs=2) as T1,
            tc.tile_pool(name="wstream", bufs=16) as WS,
            tc.tile_pool(name="wvp", bufs=8) as WV,
            tc.tile_pool(name="ptp", bufs=2) as PT,
            tc.tile_pool(name="nrm", bufs=4) as NR,
        ):
            _REPEAT = int(os.environ.get("BUILD_REPEAT", "1"))
            for _rep in range(_REPEAT):
                ones = P.tile([1, 128], bf16, tag="ones")
                nc.vector.memset(ones[:], 1.0)
                ones65 = P.tile([65, 64], f32, tag="ones65")
                nc.vector.memset(ones65[:], 1.0)
                qt = [P.tile([128, S], bf16, tag=f"qt{i}", name=f"qt{i}") for i in range(DC)]
                kt = [P.tile([128, S], bf16, tag=f"kt{i}", name=f"kt{i}") for i in range(DC)]
                vaug = [P.tile([128, HPC * 65], bf16, tag=f"va{i}", name=f"va{i}") for i in range(SC)]
                aoh = [P.tile([64, S], bf16, tag=f"aoh{i}", name=f"aoh{i}") for i in range(HPC)]
                xt = [P.tile([128, S], bf16, tag=f"xt{i}", name=f"xt{i}") for i in range(CC)]
                for i in range(CC):
                    nc.sync.dma_start(xt[i][:], xT_d.ap()[i * 128:(i + 1) * 128, :])
                cos_t = P.tile([128, S], bf16, tag="cos")
                sin_t = P.tile([128, S], bf16, tag="sin")
                nc.sync.dma_start(cos_t[:], cos_d.ap()[:])
                nc.sync.dma_start(sin_t[:], sin_d.ap()[:])
                bq4 = P.tile([128, 4], f32, tag="bq4")
                bk4 = P.tile([128, 4], f32, tag="bk4")
                bv_sb = P.tile([1, DG], bf16, tag="bv_sb")
                nc.sync.dma_start(bq4[:], bq_d.ap()[:])
                nc.sync.dma_start(bk4[:], bk_d.ap()[:])
                nc.sync.dma_start(bv_sb[:], bv_d.ap()[:])

                # ---- per-chunk: Q/K projection + rope, then attention ----
                wqk_sb = {}
                for wi, w_d in enumerate([wq_d, wk_d]):
                    wqk_sb[wi] = [WS.tile([128, DG], bf16, tag="w", name=f"w{wi}_{_}") for _ in range(CC)]
                    for i in range(CC):
                        nc.sync.dma_start(wqk_sb[wi][i][:], w_d.ap()[i * 128:(i + 1) * 128, :])
                def emit_qk(dc):
                    dsl = slice(dc * 128, (dc + 1) * 128)
                    for wi, (b4, dst) in enumerate([(bq4, qt), (bk4, kt)]):
                        w_sb = wqk_sb[wi]
                        qtsb = T1.tile([128, S], bf16, tag="qtsb", bufs=2)
                        for st in range(4):
                            sl = slice(st * 512, (st + 1) * 512)
                            ps = PS.tile([128, 512], f32, tag="proj", name="psp")
                            for cc in range(CC):
                                nc.tensor.matmul(
                                    ps[:], w_sb[cc][:, dsl], xt[cc][:, sl],
                                    start=(cc == 0), stop=(cc == CC - 1),
                                )
                            nc.vector.tensor_scalar(
                                qtsb[:, sl], ps[:], b4[:, dc:dc + 1], None,
                                op0=ALU.add,
                            )
                        qsh = T1.tile([128, S], bf16, tag="qsh", bufs=1)
                        for blk in range(2):
                            b0 = 64 * blk
                            nc.sync.dma_start(
                                qsh[b0:b0 + 32, :], qtsb[b0 + 32:b0 + 64, :]
                            )
                            nc.sync.dma_start(
                                qsh[b0 + 32:b0 + 64, :], qtsb[b0:b0 + 32, :]
                            )
                        nc.vector.tensor_tensor(dst[dc][:], qtsb[:], cos_t[:], op=ALU.mult)
                        tt2 = T1.tile([128, S], bf16, tag="tt2", bufs=1)
                        nc.vector.tensor_tensor(tt2[:], qsh[:], sin_t[:], op=ALU.mult)
                        nc.vector.tensor_tensor(dst[dc][:], dst[dc][:], tt2[:], op=ALU.add)

                def emit_attn(dc):
                    # ---- attention for head pair dc ----
                    pr = dc
                    rscoll = NR.tile([65, NPP * QT2], f32, tag="rscoll", bufs=1, name="rscoll")
                    for q in range(NQ):
                        qsl = slice(q * QT2, (q + 1) * QT2)
                        pso = [
                            PS.tile([65, QT2], f32, tag="psoA", name="psoA", bufs=1),
                            PS.tile([65, QT2], f32, tag="psoB", name="psoB", bufs=1),
                        ]
                        for ks in range(SC):
                            ksl = slice(ks * 128, (ks + 1) * 128)
                            pss = PS.tile([128, 2 * QT2], f32, tag="big", name="pss")
                            for half in range(2):
                                rows = slice(64 * half, 64 * half + 64)
                                nc.tensor.matmul(
                                    pss[:, half * QT2:(half + 1) * QT2],
                                    kt[pr][rows, ksl],
                                    qt[pr][rows, qsl],
                                    start=True, stop=True,
                                )
                            ptile = PT.tile([128, 2 * QT2], bf16, tag="pt", name="ptile", bufs=3)
                            nc.scalar.activation(ptile[:], pss[:], AF.Exp, scale=0.125)
                            for half in range(2):
                                lh = 2 * pr + half
                                nc.tensor.matmul(
                                    pso[half][:],
                                    vaug[ks][:, 65 * lh:65 * lh + 65],
                                    ptile[:, half * QT2:(half + 1) * QT2],
                                    start=(ks == 0), stop=(ks == SC - 1),
                                )
                        for half in range(2):
                            lh = 2 * pr + half
                            csl = slice((q * 2 + half) * QT2, (q * 2 + half + 1) * QT2)
                            nc.vector.tensor_copy(aoh[lh][:, qsl], pso[half][0:64, :])
                            nc.vector.tensor_copy(rscoll[64:65, csl], pso[half][64:65, :])
                    # batched denominators for this pair: one Ln + one Exp,
                    # then per-point broadcast and in-place normalize of aoh
                    nc.scalar.activation(rscoll[64:65, :], rscoll[64:65, :], AF.Ln)
                    nc.scalar.activation(
                        rscoll[64:65, :], rscoll[64:65, :], AF.Exp, scale=-1.0
                    )
                    for q in range(NQ):
                        qsl = slice(q * QT2, (q + 1) * QT2)
                        for half in range(2):
                            lh = 2 * pr + half
                            csl = slice((q * 2 + half) * QT2, (q * 2 + half + 1) * QT2)
                            psb = PS.tile([64, QT2], f32, tag="psoA", name="psb", bufs=1)
                            nc.tensor.matmul(
                                psb[:], ones65[64:65, 0:64],
                                rscoll[64:65, csl], start=True, stop=True,
                            )
                            recb = NR.tile([64, QT2], f32, tag="recb", bufs=1)
                            nc.vector.tensor_copy(recb[:], psb[:])
                            nc.vector.tensor_tensor(
                                aoh[lh][:, qsl], aoh[lh][:, qsl], recb[:],
                                op=ALU.mult,
                            )

                emit_qk(0)
                # ---- V projection first (attention needs all of it) ----
                wv_sb = [WV.tile([128, DG], bf16, tag="wv", name=f"wv_{_}") for _ in range(CC)]
                for i in range(CC):
                    nc.sync.dma_start(wv_sb[i][:], wv_d.ap()[i * 128:(i + 1) * 128, :])
                for sc in range(SC):
                    ssl = slice(sc * 128, (sc + 1) * 128)
                    ps = PS.tile([128, 512], f32, tag="proj", name="psv")
                    for cc in range(CC):
                        nc.tensor.matmul(
                            ps[:], xt[cc][:, ssl], wv_sb[cc][:],
                            start=(cc == 0), stop=False,
                        )
                    nc.tensor.matmul(
                        ps[:], ones[0:1, 0:128], bv_sb[:], start=False, stop=True,
                    )
                    va3 = vaug[sc][:].rearrange("p (h c) -> p h c", c=65)
                    ps3 = ps[:].rearrange("p (h c) -> p h c", c=64)
                    nc.vector.tensor_copy(va3[:, :, 0:64], ps3[:, :, :])
                    nc.vector.memset(va3[:, :, 64:65], 1.0)

                emit_attn(0)
                for dc in range(1, DC):
                    emit_qk(dc)
                    emit_attn(dc)

                # ---- output projection ----
                wo_sb = [P.tile([64, D], bf16, tag=f"wo{i}", name=f"wo{i}") for i in range(HPC)]
                for i in range(HPC):
                    nc.sync.dma_start(wo_sb[i][:], wo_d.ap()[i * 64:(i + 1) * 64, :])
                bo_sb = P.tile([1, D], bf16, tag="bo")
                nc.sync.dma_start(bo_sb[:], bo_d.ap()[:])
                for sc in range(SC):
                    ssl = slice(sc * 128, (sc + 1) * 128)
                    ps = PS.tile([128, 2 * QT2], f32, tag="big", name="pso3")
                    for nt in range(2):
                        nsl = slice(nt * 512, (nt + 1) * 512)
                        for h8 in range(HPC):
                            nc.tensor.matmul(
                                ps[:, nsl], aoh[h8][:, ssl], wo_sb[h8][:, nsl],
                                start=(h8 == 0), stop=False,
                            )
                        nc.tensor.matmul(
                            ps[:, nsl], ones[0:1, 0:128], bo_sb[0:1, nsl],
                            start=False, stop=True,
                        )
                    ob = T1.tile([128, D], f32, tag="qtsb", name="ob", bufs=2)
                    nc.vector.tensor_copy(ob[:], ps[:])
                    nc.sync.dma_start(out_d.ap()[ssl, :], ob[:])

    nc.compile()
    return nc


# head-local dim permutation: evens first, odds second. Q/K projection
# columns, their biases, and the rope tables all use this layout so the
# rotate-half partner of row j is row j+-32 (a contiguous block swap).
PERM64 = np.concatenate([np.arange(0, HDIM, 2), np.arange(1, HDIM, 2)])
PERMDG = np.concatenate([h * HDIM + PERM64 for h in range(HPC)])


def _rope_tables(start):
    inv_freq = 1.0 / (ROPE_BASE ** (np.arange(0, HDIM, 2, dtype=np.float64) / HDIM))
    j = np.arange(128) % HDIM
    row_freq = inv_freq[j % 32]  # [128] permuted-row frequency
    pos = np.arange(S, dtype=np.float64)
    rel = np.where(pos >= start, pos - start, 0.0)
    ang = row_freq[:, None] * rel[None, :]
    on = (pos >= start)[None, :]
    cosT = np.where(on, np.cos(ang), 1.0)
    sinT = np.where(on, np.sin(ang), 0.0)
    # evens block (j<32) pairs with +32 partner using -sin; odds block +sin
    sign = np.where(j < 32, -1.0, 1.0)
    sinTs = sinT * sign[:, None]
    return cosT, sinTs


def prepare_in_maps(inputs):
    import ml_dtypes

    bf16 = ml_dtypes.bfloat16
    x = np.asarray(inputs["x"], dtype=np.float32)
    start = int(np.asarray(inputs["rope_start_index"]))

    cosT, sinTs = _rope_tables(start)
    cosT = cosT.astype(bf16)
    sinTs = sinTs.astype(bf16)

    xTs = [np.ascontiguousarray(x[b].T).astype(bf16) for b in range(B)]

    per_hg = []
    for hg in range(HG):
        csl = slice(hg * DG, (hg + 1) * DG)
        m = {}
        for name in ("q", "k"):
            w = np.asarray(inputs["W" + name], dtype=np.float32)[:, csl][:, PERMDG]
            bvec = np.asarray(inputs["b" + name], dtype=np.float32)[csl][PERMDG]
            m["w" + name] = np.ascontiguousarray(w).astype(bf16)
            m["b" + name + "4"] = np.ascontiguousarray(
                bvec.reshape(4, 128).T
            ).astype(np.float32)
        m["wv"] = np.asarray(inputs["Wv"], dtype=np.float32)[:, csl].astype(bf16)
        m["bv"] = np.asarray(inputs["bv"], dtype=np.float32)[None, csl].astype(bf16)
        m["wo"] = np.asarray(inputs["Wo"], dtype=np.float32)[csl, :].astype(bf16)
        bo = np.asarray(inputs["bo"], dtype=np.float32)
        m["bo"] = (bo if hg == 0 else np.zeros_like(bo))[None, :].astype(bf16)
        per_hg.append(m)

    in_maps = []
    for c in range(NCORES):
        b, hg = c // HG, c % HG
        m = per_hg[hg]
        _nw = (int(os.environ.get("BUILD_REPEAT", "1"))
               + 100 * int(os.environ.get("BUILD_NONCE", "0")))
        in_maps.append({
            "nonce": np.zeros((1, _nw), np.float32),
            "xT": xTs[b],
            "wq": m["wq"], "wk": m["wk"], "wv": m["wv"], "wo": m["wo"],
            "cosT": cosT, "sinTs": sinTs,
            "bq4": m["bq4"], "bk4": m["bk4"],
            "bv": m["bv"], "bo": m["bo"],
        })
    return in_maps


def kernel(**inputs):
    from concourse.bass_utils import run_bass_kernel_spmd

    if "nc" not in _CACHE:
        _CACHE["nc"] = _build_program()
    nc = _CACHE["nc"]

    in_maps = prepare_in_maps(inputs)
    res = run_bass_kernel_spmd(nc, in_maps, core_ids=list(range(NCORES)))
    out = np.empty((B, S, D), dtype=np.float32)
    for b in range(B):
        out[b] = res.results[HG * b]["out"] + res.results[HG * b + 1]["out"]
    return out

